# revision 12
# baseline (speedup 1.0000x reference)
"""Causal multi-head attention (B=4, S=1024, D=1024, H=16, hd=64) on 8 TRN2 cores.

Sharding: head-parallel. Core c owns heads {2c, 2c+1} for all batches, i.e.
d-columns [128c, 128c+128) of q/k/v/out. Each core runs independent causal
attention for its 8 (batch, head) pairs; no collectives.

v2 design (vs v1's ACT-only exp at a 38us ACT floor): the exp work is split
across ACT and DVE, the causal mask is folded into the DVE op, and the PE
transposes + DVE PSUM->SBUF copies are replaced by xbar DMA transposes.

  - qT/kT [128d, 1024s] fp16 tiles come straight from DRAM via
    dma_start_transpose (xbar, ~896ns per [1024,128] tensor).  No identity
    matmuls, no PSUM staging, no DVE copies.
  - scoresT blocks are computed by PE into 9 PSUM groups of 1024 cols per
    batch (3-deep ring, slot<->engine affinity; spans never cross a 512-col
    PSUM bank, and every bank is written by a single head's matmuls -- HW
    rejects mixed lhsT partition bases within one bank).
  - exp: 6 groups on ACT (exact exp LUT, scale folded), 3 groups on DVE via
    a Schraudolph-style bit-trick: fp16 bits of exp(SCALE*x) are
    round(x*A + B) with A = SCALE*1024*log2(e), B = (15-sigma)*1024.  The
    DVE op is scalar_tensor_tensor: out_i16 = (x + B/A) * tri, where tri is
    a per-element constant = A off-diagonal / A*(qr>=kc) on the 6 packed
    diagonal 128-blocks per head -> masked entries become exact +0.0 and the
    int16 result is bitcast as the fp16 expT tile (verified on HW: DVE
    converts fp32->int16 with round-to-nearest; values stay in [6k,25k] so
    no wrap/saturate).  Scores are O(6) so fp32/fp16 exp cannot overflow.
  - the first two diagonal blocks per head (query rows 0..255, few-key
    softmax rows where the ~3% bit-exp error is worst) go in an ACT group
    and are masked by gpsimd affine_select (Pool is otherwise idle).
    Measured end-to-end rel err ~6e-3 vs the 2e-2 gate.
  - out[qr, hd] and the softmax denominator come from one PE accumulation:
    lhsT = expT block slices [kc, qr], rhs = v_aug [kc, 65] (v plus a ones
    column, prepared host-side).  reciprocal + normalize stay on DVE
    (gpsimd has no PSUM port).
  - PV/normalize for batch b interleave with batch b+1's exp groups; the
    out store (fp16, host upcasts) is dispatched one section later so the
    in-order SP queue never stalls input loads on a late normalize; v_aug
    for all batches loads once at start; qkT xbar-transposes are issued a
    full section ahead (4-deep ring).  Per-iter engine busy: ACT ~24.9us,
    PE ~24us, DVE ~22.6us, DMA ~16us, Pool ~3us; TimelineSim single-shot
    38.7us / steady 25.1us vs the v1 baseline's 53.4/38.6 (HW-measured
    steady on a quiet machine: 24.8us/iter).

TRN2 instructions have one HW semaphore-wait slot; split_multi_waits()
legalizes multi-producer waits.
"""

import sys

sys.path.insert(0, "/opt/trn_rl_repo")

import numpy as np

import concourse.bass as bass
import concourse.mybir as mybir
import concourse.tile as tile
from concourse import bass_utils

B, S, D, H = 4, 1024, 1024, 16
HD = 64
NCORES = 8
HPC = H // NCORES          # heads per core = 2
CW = HPC * HD              # per-core d-column width = 128
P = 128                    # partitions
NT = S // P                # 8 s-tiles of 128
GCOLS = 1024               # psum exp-group width (2 banks)
NG = 9                     # exp groups per batch
SCALE = HD ** -0.5
FP32 = mybir.dt.float32
F16 = mybir.dt.float16
I16 = mybir.dt.int16
F16_NP = np.float16

# bit-exp constants: fp16 bits of exp(SCALE*x) ~= round(x*AEXP + BEXP)
SIGMA = 0.02
AEXP = float(SCALE * 1024.0 * np.log2(np.e))
BEXP = (15.0 - SIGMA) * 1024.0
BA = BEXP / AEXP

# off-diagonal span of score block (c, j): qr in [qs, qs+w), kc block j
_OFF = {(0, 0): (128, 384), (0, 1): (256, 256), (0, 2): (384, 128),
        (1, 0): (512, 512), (1, 1): (512, 512), (1, 2): (512, 512),
        (1, 3): (512, 512), (1, 4): (640, 384), (1, 5): (768, 256),
        (1, 6): (896, 128)}


def split_multi_waits(nc):
    """TRN2 TPB instructions carry exactly one semaphore wait slot; walrus
    refuses >1 on_wait per instruction.  Hoist extra waits onto standalone
    EventSemaphore instructions on the same engine, inserted right before the
    owning instruction (engines dispatch in order, so semantics are
    unchanged)."""
    ctr = [0]
    for fn in nc.m.functions:
        for blk in fn.blocks:
            insts = list(blk.instructions)
            out = []
            changed = False
            for inst in insts:
                si = inst.sync_info
                if si is not None and len(si.on_wait) > 1:
                    changed = True
                    waits = list(si.on_wait)
                    for w in waits[:-1]:
                        ev = mybir.InstEventSemaphore(
                            name=f"evw-split-{ctr[0]}", ins=[], outs=[]
                        )
                        ctr[0] += 1
                        ev.engine = inst.engine
                        ev.sync_info = mybir.SyncInfo(on_wait=[w], on_update=[])
                        out.append(ev)
                    inst.sync_info = mybir.SyncInfo(
                        on_wait=[waits[-1]], on_update=list(si.on_update)
                    )
                out.append(inst)
            if changed:
                for i, inst in enumerate(out):
                    existing = blk.instructions
                    if i < len(existing) and existing[i].name == inst.name:
                        continue
                    blk.instructions.insert(i, inst)


_FLIP = None


def _schedule():
    """Per-batch exp-group schedule (identical for all batches).

    v3: every 1024-col psum tile is split (h0 bin | h1 bin) at 512 cols, the
    same 512-col bin structure for both heads.  The QK matmuls of the two
    bins are emitted INTERLEAVED: consecutive instructions alternate lhsT
    partition base 0/64, i.e. PE row groups 0-1 vs 2-3, which the HW runs
    concurrently (row-tiled matmuls; microbenched ~2x).  Banks stay
    head-homogeneous (h0 = bank 0, h1 = bank 1 of each tile) which the HW
    requires (all matmuls writing one 512-col bank share the lhsT base).

    Bin layout per head (goff within the 512-col bin; h1 adds 512):
      B0: diag qt0@0, qt1@128 (exact exp + Pool select), off(0,2)@256,
          off(1,6)@384                                          -> ACT
      B1: diag qt2..qt5 @ 0,128,256,384 (bit-exp, tri mask)     -> DVE tri1
      B2: diag qt6,qt7 @ 0,128 (tri), off(0,1)@256 w256         -> DVE tri2
      B3..B6: off(1,0..3) w512                                  -> free
      B7: off(1,4)@0 w384, off(1,5)[768:896]@384                -> free
      B8: off(0,0)@0 w384, off(1,5)[896:1024]@384               -> free
    Free tiles run on ACT (exact exp) or DVE (plain bit-exp) per the
    iteration-level balance: ACT 23 tiles/iter, DVE 13 (incl forced tri).

    Returns (orders, xs) where xs[(hl, qt, j)] = expT col of the 128-wide
    qr-slice [128qt..128qt+128) of key-block j (j == qt is the diagonal).
    """
    def off(hl, c, j, goff, qs=None, w=None):
        qs0, w0 = _OFF[(c, j)]
        return (hl, c, j, qs0 if qs is None else qs, w0 if w is None else w,
                goff)

    def diag(hl, qt, goff):
        return (hl, qt // 4, qt, 128 * qt, 128, goff)

    def bins(hl):
        return [
            dict(kind="act0", sel=[0, 128], spans=[
                diag(hl, 0, 0), diag(hl, 1, 128), off(hl, 0, 2, 256),
                off(hl, 1, 6, 384)]),
            dict(kind="tri1", sel=[], spans=[
                diag(hl, qt, 128 * (qt - 2)) for qt in range(2, 6)]),
            dict(kind="tri2", sel=[], spans=[
                diag(hl, 6, 0), diag(hl, 7, 128), off(hl, 0, 1, 256)]),
            dict(kind="free", sel=[], spans=[off(hl, 1, 0, 0)]),
            dict(kind="free", sel=[], spans=[off(hl, 1, 1, 0)]),
            dict(kind="free", sel=[], spans=[off(hl, 1, 2, 0)]),
            dict(kind="free", sel=[], spans=[off(hl, 1, 3, 0)]),
            dict(kind="free", sel=[], spans=[
                off(hl, 1, 4, 0), off(hl, 1, 5, 384, qs=768, w=128)]),
            dict(kind="free", sel=[], spans=[
                off(hl, 0, 0, 0), off(hl, 1, 5, 384, qs=896, w=128)]),
        ]

    b0, b1 = bins(0), bins(1)
    tiles = []
    for t in range(9):
        spans = []
        # interleave h0/h1 spans; identical bin structure pairs equal widths
        s0, s1 = b0[t]["spans"], b1[t]["spans"]
        for k in range(max(len(s0), len(s1))):
            if k < len(s0):
                spans.append(s0[k])
            if k < len(s1):
                (hl, c, j, qs, w, goff) = s1[k]
                spans.append((hl, c, j, qs, w, goff + 512))
        sel = b0[t]["sel"] + [g + 512 for g in b1[t]["sel"]]
        tiles.append(dict(kind=b0[t]["kind"], sel=sel, spans=spans))

    T = tiles
    # Emission order keeps the psum ring's slot<->engine affinity (ring 3,
    # 9 tiles: positions 0..8 consume as [ACT, DVE, ACT] repeating), so each
    # slot recycles on a single engine's cadence and the PE never stalls on
    # the slower engine's slot.
    order_idx = [0, 1, 3, 4, 2, 5, 6, 8, 7]
    # Engine of each tile, per batch phase.  ACT 23/iter, DVE 13/iter:
    # batches 0-2: free tiles 3,4,5,6,7 on ACT, 8 on DVE;
    # batch 3:     free tiles 3,4,5,6   on ACT, 7 and 8 on DVE.
    def engines(bi):
        eng = {}
        for t in range(9):
            k = T[t]["kind"]
            if k == "act0":
                eng[t] = "act"
            elif k in ("tri1", "tri2"):
                eng[t] = k
            else:
                eng[t] = "dve" if (t == 8 or (t == _FLIP and bi == 3)) else "act"
        return eng

    # drain order for the last batch: the tiles PV half-0 (qt 0..3) needs
    # (T0: qt0/1 + (0,2); T1: qt2/3; T2: (0,1); T8: (0,0)) go first.
    final_idx = [0, 1, 2, 8, 3, 4, 5, 6, 7]

    xs = {}
    for pos, t in enumerate(order_idx):
        T[t]["xbase"] = pos * GCOLS
        T[t]["order_pos"] = pos
    for t in range(9):
        for (hl, c, j, qs, w, goff) in T[t]["spans"]:
            assert w % P == 0 and qs % P == 0
            assert goff % P == 0 and (goff % 512) + w <= 512
            for k in range(w // P):
                xs[(hl, qs // P + k, j)] = T[t]["xbase"] + goff + k * P
    for hl in range(HPC):
        for qt in range(NT):
            for j in range(qt + 1):
                assert (hl, qt, j) in xs, (hl, qt, j)
    orders = dict(order=order_idx, final=final_idx, engines=engines,
                  tiles=T)
    return orders, xs


_SCHED, _XS = _schedule()


def build_program(repeat: int = 1, debug_stage: int = 4):
    # debug_stage: 1=loads only, 2=+exp (dump expT), 3=+pv via SP store, 4=full
    nc = bass.Bass(trn_type="TRN2")
    qk_d = nc.dram_tensor("qk", [B, 2, S, CW], F16, kind="ExternalInput")
    # value_aug is host-prepacked to the exact SBUF layout for ALL batches
    # ([P, B*HPC*NT*(HD+1)]) and loaded once at program start: per-batch
    # DMAs then reduce to qkT + out, which keeps the HWDGE lane-order waits
    # off the batch critical path.
    va_d = nc.dram_tensor("value_aug", [P, B * HPC * NT * (HD + 1)], F16,
                          kind="ExternalInput")
    # fp16 output (host upcasts): halves the store on the serial DMA chain.
    o_d = nc.dram_tensor("attn_out", [B, S, CW], F16, kind="ExternalOutput")
    if debug_stage < 3:
        dbg_d = nc.dram_tensor("dbg", [P, NG * GCOLS], F16, kind="ExternalOutput")

    with tile.TileContext(nc) as tc:
        with (
            tc.tile_pool(name="const", bufs=1) as constp,
            tc.tile_pool(name="trp", bufs=4) as trp,
            tc.tile_pool(name="expp", bufs=2) as expp,
            tc.tile_pool(name="outp", bufs=3) as outp,
            tc.tile_pool(name="smallp", bufs=4) as smallp,
            tc.tile_pool(name="psc", bufs=3, space="PSUM") as psc,
            tc.tile_pool(name="pout", bufs=2, space="PSUM") as pout,
        ):
            # tri masks are generated on-chip by the (idle) Pool engine during
            # the DMA fill: memset to A, then zero the causal triangles.
            # tri1 masks all 8 diag blocks (tile T1 = qt2..5 for both heads);
            # tri2 masks blocks 0,1 of each 512-bin (tile T2 = qt6,qt7).
            tri1 = constp.tile([P, GCOLS], F16)
            tri2 = constp.tile([P, GCOLS], F16)
            nc.gpsimd.memset(tri1[:], AEXP)
            nc.gpsimd.memset(tri2[:], AEXP)
            for t in range(8):
                nc.gpsimd.affine_select(
                    out=tri1[:, P * t:P * (t + 1)],
                    in_=tri1[:, P * t:P * (t + 1)],
                    compare_op=mybir.AluOpType.is_ge,
                    fill=0.0, base=0, pattern=[[1, P]],
                    channel_multiplier=-1,
                )
            for t in (0, 1, 4, 5):
                nc.gpsimd.affine_select(
                    out=tri2[:, P * t:P * (t + 1)],
                    in_=tri2[:, P * t:P * (t + 1)],
                    compare_op=mybir.AluOpType.is_ge,
                    fill=0.0, base=0, pattern=[[1, P]],
                    channel_multiplier=-1,
                )
            va_sb = constp.tile([P, B * HPC * NT * (HD + 1)], F16)
            # Dummy 1-col exp issued first: the ~2.7us exp table-set load
            # happens during the DMA fill instead of before the first real
            # exp.  Reads whatever is in the (unloaded) tri tile; result
            # discarded into scratch.
            warm = constp.tile([P, 1], FP32)
            nc.scalar.activation(warm[:], tri1[:, 0:1],
                                 mybir.ActivationFunctionType.Exp)
            # PE p-state pre-ramp: ~3.8us of dummy matmuls (at the cold
            # 0.65/1.2GHz clocks) complete inside the ~5us DMA fill, so the
            # first real QK runs at the full 2.4GHz instead of restarting
            # the 3us ramp.
            warm_mm = constp.tile([P, 512], F16)
            nc.vector.memset(warm_mm[:], 0.0)
            warm_ps = psc.tile([P, GCOLS], FP32, tag="ps", name="warm_ps")
            for _ in range(8):
                nc.tensor.matmul(warm_ps[:, 0:512], warm_mm[0:64, 0:128],
                                 warm_mm[0:64, 0:512], start=True, stop=True)

            def emit_pv_chunk(ctx, out_sb, hl, c):
                b, expT = ctx
                v_view = va_sb.rearrange(
                    "p (v h j e) -> p v h j e", v=B, h=HPC, e=HD + 1)[:, b, hl]
                po4 = pout.tile([P, 4 * (HD + 1)], FP32, tag="po")
                for qi in range(4):
                    qt = 4 * c + qi
                    for j in range(qt + 1):
                        o = _XS[(hl, qt, j)]
                        nc.tensor.matmul(
                            po4[:, qi * (HD + 1):(qi + 1) * (HD + 1)],
                            expT[:, o:o + P],
                            v_view[:, j, :],
                            start=(j == 0), stop=(j == qt),
                        )
                po_v = po4.rearrange("p (t e) -> p t e", e=HD + 1)
                recip4 = smallp.tile([P, 4], FP32, tag="recip")
                rv = recip4.rearrange("p (t o) -> p t o", o=1)
                nc.vector.reciprocal(rv, po_v[:, :, HD:HD + 1])
                out_v = out_sb.rearrange("p (t j) -> p t j", j=P)[
                    :, c * 4:(c + 1) * 4, hl * HD:(hl + 1) * HD]
                nc.vector.tensor_mul(
                    out_v, po_v[:, :, 0:HD], rv.broadcast_to((P, 4, HD)))

            def emit_pv(ctx):
                b, expT = ctx
                out_sb = outp.tile([P, S], F16, tag="out_sb")
                for hl, c in ((0, 0), (0, 1), (1, 0), (1, 1)):
                    emit_pv_chunk(ctx, out_sb, hl, c)
                return (b, out_sb)

            def emit_out(ctx):
                # dispatched one section later, after the next batch's input
                # loads: by then the normalize is done, so the in-order SP
                # queue never stalls loads behind an out store (and the Pool
                # queue stays free for the selects, which gate PV).
                b, out_sb = ctx
                nc.sync.dma_start(
                    o_d[b].rearrange("(t p) j -> p t j", p=P),
                    out_sb.rearrange("p (t j) -> p t j", j=CW),
                )

            def load_qkT(b):
                # one xbar transpose loads q and k: [2*S, CW] -> [CW, 2*S]
                qkT = trp.tile([P, 2 * S], F16, tag="qkT", name="qkT")
                nc.sync.dma_start_transpose(
                    qkT, qk_d[b].rearrange("x s j -> (x s) j"))
                return qkT

            prev = None
            prev_out = None
            pending_qkT = load_qkT(0)
            for b_rep in range(repeat * B):
                b = b_rep % B
                qkT = pending_qkT
                qT = qkT[:, 0:S]
                kT = qkT[:, S:2 * S]
                # qkT for the NEXT batch is issued a full section early (ring
                # of 4): its transfer and the 900ns DMA-completion semaphore
                # propagation fully overlap the previous batch's compute.
                if b_rep + 1 < repeat * B:
                    pending_qkT = load_qkT((b_rep + 1) % B)
                if b_rep == 0:
                    nc.sync.dma_start(va_sb, va_d[:])
                if prev_out is not None:
                    emit_out(prev_out)
                    prev_out = None
                expT = expp.tile([P, NG * GCOLS], F16, tag="expT")
                dbg_ops = globals().get("_DBG_OPS", ("act", "dve", "sel"))
                # PV/normalize chunks of the previous batch interleave with
                # this batch's exp groups so the norms (and thus the out
                # store's SP dispatch) complete mid-section instead of at
                # the end, keeping the next qkT load unblocked.
                do_pv = debug_stage >= 3 and prev is not None
                pv_out_sb = None
                if do_pv:
                    pv_out_sb = outp.tile([P, S], F16, tag="out_sb",
                                          name="out_sb")
                pv_chunks = [(0, 0), (0, 1), (1, 0), (1, 1)]
                tiles_T = _SCHED["tiles"]
                eng_map = _SCHED["engines"](b_rep % B)
                is_final = b_rep == repeat * B - 1
                idx_order = _SCHED["final" if is_final else "order"]
                for gi, t in enumerate(idx_order):
                    g = tiles_T[t]
                    eng = eng_map[t]
                    xb = g["xbase"]
                    ps = psc.tile([P, GCOLS], FP32, tag="ps")
                    # h0/h1 spans interleaved: consecutive matmuls alternate
                    # lhsT partition base 0/64 -> PE row groups 0-1 / 2-3 run
                    # concurrently on HW.
                    for (hl, c, j, qs, w, goff) in g["spans"]:
                        hp = hl * HD
                        nc.tensor.matmul(
                            ps[:, goff:goff + w],
                            kT[hp:hp + HD, j * P:(j + 1) * P],
                            qT[hp:hp + HD, qs:qs + w],
                            start=True, stop=True,
                        )
                    if eng == "tri1" or eng == "tri2":
                        if "dve" not in dbg_ops:
                            continue
                        nc.vector.scalar_tensor_tensor(
                            expT[:, xb:xb + GCOLS].bitcast(I16),
                            ps[:], BA, (tri1 if eng == "tri1" else tri2)[:],
                            mybir.AluOpType.add, mybir.AluOpType.mult,
                        )
                    elif eng == "dve":
                        if "dve" not in dbg_ops:
                            continue
                        nc.vector.tensor_scalar(
                            expT[:, xb:xb + GCOLS].bitcast(I16),
                            ps[:], AEXP, BEXP,
                            mybir.AluOpType.mult, mybir.AluOpType.add,
                        )
                    else:
                        if "act" not in dbg_ops:
                            continue
                        nc.scalar.activation(
                            expT[:, xb:xb + GCOLS], ps[:],
                            mybir.ActivationFunctionType.Exp, scale=SCALE,
                        )
                        if "sel" not in dbg_ops:
                            continue
                        for so in g["sel"]:
                            nc.gpsimd.affine_select(
                                out=expT[:, xb + so:xb + so + P],
                                in_=expT[:, xb + so:xb + so + P],
                                compare_op=mybir.AluOpType.is_ge,
                                fill=0.0, base=0, pattern=[[1, P]],
                                channel_multiplier=-1,
                            )
                    if do_pv and gi in (2, 4, 6, 8):
                        hl, c = pv_chunks[gi // 2 - 1]
                        emit_pv_chunk(prev, pv_out_sb, hl, c)
                if do_pv:
                    prev_out = (prev[0], pv_out_sb)
                prev = (b, expT)
                if debug_stage < 3:
                    break
            if debug_stage <= 2:
                if debug_stage == 2:
                    nc.sync.dma_start(dbg_d[:], expT[:])
                else:
                    nc.sync.dma_start(dbg_d[:, 0:2 * S], qkT[:])
            else:
                if prev_out is not None:
                    emit_out(prev_out)
                # drain: final batch's PV in qr-halves, each half's store
                # overlapping the other half's compute.
                b_l = prev[0]
                fin_sb = outp.tile([P, S], F16, tag="out_sb", name="out_sb")
                ov = o_d[b_l].rearrange("(t p) j -> p t j", p=P)
                sv = fin_sb.rearrange("p (t j) -> p t j", j=CW)
                for half in range(2):
                    for hl in range(HPC):
                        emit_pv_chunk(prev, fin_sb, hl, half)
                    nc.sync.dma_start(ov[:, half * 4:(half + 1) * 4],
                                      sv[:, half * 4:(half + 1) * 4])
    split_multi_waits(nc)
    return nc


def make_in_maps(query, key, value):
    query = np.asarray(query, dtype=np.float32)
    key = np.asarray(key, dtype=np.float32)
    value = np.asarray(value, dtype=np.float32)
    in_maps = []
    for c in range(NCORES):
        sl = slice(c * CW, (c + 1) * CW)
        # packed v_aug: [P, ((b*2 + hl)*8 + j)*65 + e] = v[b, 128j+p, 64hl+e],
        # ones at e=64
        v_shard = value[:, :, sl].reshape(B, NT, P, HPC, HD)
        v_aug = np.ones((P, B, HPC, NT, HD + 1), dtype=F16_NP)
        v_aug[..., :HD] = v_shard.transpose(2, 0, 3, 1, 4).astype(F16_NP)
        qk = np.stack([query[:, :, sl], key[:, :, sl]], axis=1).astype(F16_NP)
        in_maps.append(
            {
                "qk": np.ascontiguousarray(qk),
                "value_aug": v_aug.reshape(P, B * HPC * NT * (HD + 1)),
            }
        )
    return in_maps


_RUNNER = None


def _get_runner():
    """Build the Bass program once and return a cached jitted 8-core runner
    (mirrors bass2jax.run_bass_via_pjrt's shard_map path; re-invoking
    run_bass_kernel_spmd would re-trace and re-jit on every call)."""
    global _RUNNER
    if _RUNNER is not None:
        return _RUNNER
    import jax
    from jax.sharding import Mesh, PartitionSpec
    from jax.experimental.shard_map import shard_map
    from concourse import bass2jax

    nc = build_program()
    bass2jax.install_neuronx_cc_hook()

    partition_name = nc.partition_id_tensor.name if nc.partition_id_tensor else None
    in_names, out_names, out_avals, zero_outs = [], [], [], []
    for alloc in nc.m.functions[0].allocations:
        if not isinstance(alloc, mybir.MemoryLocationSet):
            continue
        name = alloc.memorylocations[0].name
        if alloc.kind == "ExternalInput":
            if name != partition_name:
                in_names.append(name)
        elif alloc.kind == "ExternalOutput":
            shape = tuple(alloc.tensor_shape)
            dtype = mybir.dt.np(alloc.dtype)
            out_names.append(name)
            out_avals.append(jax.core.ShapedArray(shape, dtype))
            zero_outs.append(np.zeros(shape, dtype))
    n_params = len(in_names)
    all_in_names = list(in_names) + list(out_names)
    if partition_name is not None:
        all_in_names.append(partition_name)

    def _body(*args):
        operands = list(args)
        if partition_name is not None:
            operands.append(bass2jax.partition_id_tensor())
        outs = bass2jax._bass_exec_p.bind(
            *operands,
            out_avals=tuple(out_avals),
            in_names=tuple(all_in_names),
            out_names=tuple(out_names),
            lowering_input_output_aliases=(),
            sim_require_finite=True,
            sim_require_nnan=True,
            nc=nc,
        )
        return tuple(outs)

    devices = jax.devices()[:NCORES]
    mesh = Mesh(np.asarray(devices), ("core",))
    spec = PartitionSpec("core")
    fn = jax.jit(
        shard_map(_body, mesh=mesh,
                  in_specs=(spec,) * (n_params + len(out_names)),
                  out_specs=(spec,) * len(out_names), check_rep=False),
        keep_unused=True,
    )
    _RUNNER = (fn, in_names, out_names, out_avals, zero_outs)
    return _RUNNER


def _concat_inputs(query, key, value):
    """Vectorized equivalent of concatenating make_in_maps() over cores:
    returns {name: [(8*dim0), ...] array} keyed like the ExternalInputs."""
    q16 = np.asarray(query, dtype=F16_NP).reshape(B, S, NCORES, CW)
    k16 = np.asarray(key, dtype=F16_NP).reshape(B, S, NCORES, CW)
    # qk: per core [B, 2, S, CW] -> concat [(8B), 2, S, CW]
    qk = np.stack(
        [q16.transpose(2, 0, 1, 3), k16.transpose(2, 0, 1, 3)], axis=2
    ).reshape(NCORES * B, 2, S, CW)
    v16 = np.asarray(value, dtype=F16_NP).reshape(B, NT, P, NCORES, HPC, HD)
    v_aug = np.ones((NCORES, P, B, HPC, NT, HD + 1), dtype=F16_NP)
    v_aug[..., :HD] = v16.transpose(3, 2, 0, 4, 1, 5)
    v_aug = v_aug.reshape(NCORES * P, B * HPC * NT * (HD + 1))
    return {
        "qk": np.ascontiguousarray(qk),
        "value_aug": np.ascontiguousarray(v_aug),
    }


def kernel(query: np.ndarray, key: np.ndarray, value: np.ndarray) -> np.ndarray:
    fn, in_names, out_names, out_avals, zero_outs = _get_runner()
    cat = _concat_inputs(query, key, value)
    concat_in = [cat[name] for name in in_names]
    concat_zeros = [
        np.zeros((NCORES * z.shape[0], *z.shape[1:]), z.dtype) for z in zero_outs
    ]
    out_arrs = fn(*concat_in, *concat_zeros)
    oi = out_names.index("attn_out")
    full = np.asarray(out_arrs[oi]).reshape(NCORES, *out_avals[oi].shape)
    return np.concatenate(list(full), axis=2).astype(np.float32)

